# revision 12
# baseline (speedup 1.0000x reference)
"""Trainium2 Bass kernel for nn_Loss_29111288332476 (segment_reduce).

Computes, for features [131072,512] f32, center [1000,512] f32, labels [131072] int:
    thres      = 3 * mean(cdist(center, center))
    counts[c]  = #{i : labels[i] == c}
    sums[c]    = sum_{i: labels[i]==c} features[i]
    new_center = (center + sums) / max(counts, 1)
    loss       = mean(relu(thres - cdist(new_center, new_center)))
returns (loss, new_center).

Strategy: data-parallel over batch on 8 cores. Each core:
  - counts via a per-tile [128b x 128c_lo] x [128b x 8c_hi] bf16 one-hot matmul
    accumulated in one PSUM bank,
  - local segment sums via one-hot matmuls, feature-stationary orientation:
    psum[d_chunk, c_half] += feat[:, d_chunk].T @ onehot[:, c_half] (float32r),
    accumulated across all 128 batch tiles in 8 PSUM banks -> table_T [512, 1024],
  - AllReduce of [513, 1024] (sums_T + counts row) across cores,
  - replicated epilogue: divide by counts, two cdist passes on PE, hinge mean.
"""

import sys

for p in ("/opt/trn_rl_repo", "/opt/trn_rl_repo/concourse"):
    if p not in sys.path:
        sys.path.insert(0, p)

import numpy as np

import concourse.bass as bass
import concourse.bacc as bacc
import concourse.tile as tile
import concourse.mybir as mybir
from concourse.bass_utils import run_bass_kernel_spmd
from concourse.masks import make_identity

P = 128
D = 512
C = 1000
CP = 1024           # classes padded to 8*128
NH = CP // P        # 8 class chunks
ND = D // P         # 4 feature chunks
NCORES = 8
B = 131072
BL = B // NCORES    # 16384 rows per core
NT = BL // P        # 128 batch tiles per core

f32 = mybir.dt.float32
f32r = mybir.dt.float32r
bf16 = mybir.dt.bfloat16
i32 = mybir.dt.int32
OP = mybir.AluOpType
ACT = mybir.ActivationFunctionType

# j-halves of the class dim restricted to the 1000 real classes
JSPANS = [(0, 512), (512, 488)]
# i-chunks: 7 full + 1 partial (classes 896..999)
ISIZES = [128] * 7 + [C - 7 * 128]


def _r(ap):
    return ap.bitcast(f32r)


def _build():
    nc = bacc.Bacc("TRN2", target_bir_lowering=False, debug=False,
                   num_devices=NCORES)

    feat_d = nc.dram_tensor("feat", [BL, D], f32, kind="ExternalInput")
    lab_d = nc.dram_tensor("labels", [BL], i32, kind="ExternalInput")
    cen_d = nc.dram_tensor("center", [C, D], f32, kind="ExternalInput")
    outc_d = nc.dram_tensor("out_center", [C, D], f32, kind="ExternalOutput")
    outp_d = nc.dram_tensor("out_partial", [1, 1], f32, kind="ExternalOutput")

    with tile.TileContext(nc) as tc:
        with (
            tc.tile_pool(name="const", bufs=1) as cp,
            tc.tile_pool(name="feat", bufs=4) as fp,
            tc.tile_pool(name="oh", bufs=3) as ohp,
            tc.tile_pool(name="big", bufs=1) as bigp,
            tc.tile_pool(name="small", bufs=2) as smp,
            tc.tile_pool(name="dram", bufs=1, space="DRAM") as dp,
        ):
            ident = cp.tile([P, P], f32)
            make_identity(nc, ident[:])
            ones_col = cp.tile([P, 1], f32)
            nc.vector.memset(ones_col[:], 1.0)
            ones11 = cp.tile([1, 1], f32)
            nc.vector.memset(ones11[:], 1.0)

            iota_full = cp.tile([P, CP], f32)
            nc.gpsimd.iota(iota_full[:], pattern=[[1, CP]], base=0,
                           channel_multiplier=0,
                           allow_small_or_imprecise_dtypes=True)
            iota_lo = cp.tile([P, P], f32)
            nc.gpsimd.iota(iota_lo[:], pattern=[[1, P]], base=0,
                           channel_multiplier=0,
                           allow_small_or_imprecise_dtypes=True)
            iota_hi = cp.tile([P, NH], f32)
            nc.gpsimd.iota(iota_hi[:], pattern=[[P, NH]], base=0,
                           channel_multiplier=0,
                           allow_small_or_imprecise_dtypes=True)

            # labels: lab_sb[p, t] = labels[t*128 + p]
            lab_i = cp.tile([P, NT], i32)
            nc.sync.dma_start(out=lab_i[:],
                              in_=lab_d[:].rearrange("(t p) -> p t", p=P))
            lab_lo_i = cp.tile([P, NT], i32)
            nc.vector.tensor_scalar(lab_lo_i[:], lab_i[:], 127, None,
                                    OP.bitwise_and)
            lab_hb_i = cp.tile([P, NT], i32)  # label - label%128 = 128*(label>>7)
            nc.vector.tensor_tensor(out=lab_hb_i[:], in0=lab_i[:],
                                    in1=lab_lo_i[:], op=OP.subtract)
            lab_sb = cp.tile([P, NT], f32)
            nc.vector.tensor_copy(out=lab_sb[:], in_=lab_i[:])
            lab_lo = cp.tile([P, NT], f32)
            nc.vector.tensor_copy(out=lab_lo[:], in_=lab_lo_i[:])
            lab_hb = cp.tile([P, NT], f32)
            nc.vector.tensor_copy(out=lab_hb[:], in_=lab_hb_i[:])

            # local table bounce (for AllReduce): rows 0..1023 = sums [c, d],
            # rows 1024..1025 = counts (flattened [128,8] c_lo-major)
            loc_tab = dp.tile([CP + 2, D], f32)
            shr_tab = dp.tile([CP + 2, D], f32, addr_space="Shared")

            # ---------------- phase A: counts ----------------
            with tc.tile_pool(name="pscnt", bufs=1, space="PSUM") as pcp:
                cnt_ps = pcp.tile([P, NH], f32)
                for t in range(NT):
                    oh_lo = ohp.tile([P, P], bf16, tag="ohlo")
                    nc.vector.tensor_scalar(oh_lo[:], iota_lo[:],
                                            lab_lo[:, t:t + 1], None,
                                            OP.is_equal)
                    oh_hi = ohp.tile([P, NH], bf16, tag="ohhi")
                    nc.vector.tensor_scalar(oh_hi[:], iota_hi[:],
                                            lab_hb[:, t:t + 1], None,
                                            OP.is_equal)
                    nc.tensor.matmul(cnt_ps[:], lhsT=oh_lo[:], rhs=oh_hi[:],
                                     start=(t == 0), stop=(t == NT - 1))
                cnt_sb = cp.tile([P, NH], f32)
                nc.vector.tensor_copy(out=cnt_sb[:], in_=cnt_ps[:])
            nc.sync.dma_start(
                out=loc_tab[CP:CP + 2, :].flatten().rearrange("(p h) -> p h", p=P),
                in_=cnt_sb[:])

            # ---------------- phase B: segment sums ----------------
            with tc.tile_pool(name="psmain", bufs=1, space="PSUM") as pmp:
                banks = [pmp.tile([P, 512], f32, tag=f"bank{i}",
                                  name=f"bank{i}")
                         for i in range(ND * 2)]
                for t in range(NT):
                    ft = fp.tile([P, D], f32r, tag="ft")
                    nc.sync.dma_start(out=ft[:],
                                      in_=_r(feat_d[t * P:(t + 1) * P, :]))
                    oh = ohp.tile([P, CP], f32r, tag="oh")
                    nc.vector.tensor_scalar(oh[:], iota_full[:],
                                            lab_sb[:, t:t + 1], None,
                                            OP.is_equal)
                    for h in range(NH):
                        nc.tensor.matmul(
                            banks[h][:],
                            lhsT=oh[:, h * P:(h + 1) * P],
                            rhs=ft[:],
                            start=(t == 0), stop=(t == NT - 1))
                stage = bigp.tile([P, NH * D], f32, tag="stage")
                for h in range(NH):
                    nc.vector.tensor_copy(
                        out=stage[:, h * D:(h + 1) * D],
                        in_=banks[h][:])
                for h in range(NH):
                    nc.sync.dma_start(
                        out=loc_tab[h * P:(h + 1) * P, :],
                        in_=stage[:, h * D:(h + 1) * D])

            # ---------------- AllReduce ----------------
            nc.gpsimd.collective_compute(
                "AllReduce", OP.add,
                replica_groups=[list(range(NCORES))],
                ins=[loc_tab.opt()], outs=[shr_tab.opt()])

            # ------------- phase D: init cdist (center only; overlaps AR) ----
            cen_sb = bigp.tile([P, NH * D], f32, tag="cen")   # [c_lo, h*512+d]
            nc.vector.memset(cen_sb[:], 0.0)
            for h in range(NH):
                rows = min(C - h * P, P)
                if rows <= 0:
                    break
                nc.sync.dma_start(out=cen_sb[:rows, h * D:h * D + D],
                                  in_=cen_d[h * P:h * P + rows, :])
            with tc.tile_pool(name="pst", bufs=1, space="PSUM") as ptp:
                cenT = bigp.tile([P, ND * CP], f32, tag="cenT")  # [d_lo, dh*1024+c]
                for h in range(NH):
                    for dh in range(ND):
                        ps = ptp.tile([P, P], f32, tag="tps", bufs=3)
                        nc.tensor.transpose(
                            ps[:], cen_sb[:, h * D + dh * P:h * D + dh * P + P],
                            ident[:])
                        nc.vector.tensor_copy(
                            out=cenT[:, dh * CP + h * P:dh * CP + h * P + P],
                            in_=ps[:])

                cenT_r = bigp.tile([P, ND * CP], f32r, tag="cenT_r")
                nc.vector.tensor_copy(out=cenT_r[:], in_=cenT[:])

                # n2 for init centers: column [c_lo, h] then broadcast row
                n2i = smp.tile([P, NH], f32, tag="n2i")
                sqt = smp.tile([P, D], f32, tag="sqt")
                for h in range(NH):
                    nc.vector.tensor_tensor(out=sqt[:], in0=cen_sb[:, h * D:h * D + D],
                                            in1=cen_sb[:, h * D:h * D + D], op=OP.mult)
                    nc.vector.tensor_reduce(out=n2i[:, h:h + 1], in_=sqt[:],
                                            axis=mybir.AxisListType.X, op=OP.add)
                n2ib = bigp.tile([P, CP], f32, tag="n2ib")    # row-broadcast
                for h in range(NH):
                    ps = ptp.tile([P, P], f32, tag="tps", bufs=3)
                    nc.tensor.transpose(ps[:], n2i[:, h:h + 1].to_broadcast([P, P]),
                                        ident[:])
                    nc.vector.tensor_copy(out=n2ib[:, h * P:h * P + P], in_=ps[:])

                # G tiles + dist sums
                dsum = smp.tile([P, 16], f32, tag="dsum")
                nc.vector.memset(dsum[:], 0.0)
                dtile = smp.tile([P, 512], f32, tag="dtile")
                for ih in range(NH):
                    m = ISIZES[ih]
                    for jh, (j0, jn) in enumerate(JSPANS):
                        g = ptp.tile([P, 512], f32, tag="gps", bufs=3)
                        for dh in range(ND):
                            nc.tensor.matmul(
                                g[:m, :jn],
                                lhsT=cenT_r[:, dh * CP + ih * P:dh * CP + ih * P + m],
                                rhs=cenT_r[:, dh * CP + j0:dh * CP + j0 + jn],
                                start=(dh == 0), stop=(dh == ND - 1))
                        # d2 = n2_i - 2G + n2_j ; dist = sqrt(max(d2,0))
                        nc.vector.tensor_scalar(
                            dtile[:m, :jn], g[:m, :jn], -2.0, n2i[:m, ih:ih + 1],
                            OP.mult, op1=OP.add)
                        nc.vector.tensor_tensor(out=dtile[:m, :jn],
                                                in0=dtile[:m, :jn],
                                                in1=n2ib[:m, j0:j0 + jn], op=OP.add)
                        nc.vector.tensor_scalar(dtile[:m, :jn], dtile[:m, :jn],
                                                0.0, None, OP.max)
                        nc.scalar.activation(dtile[:m, :jn], dtile[:m, :jn],
                                             ACT.Sqrt,
                                             accum_out=dsum[:m, ih * 2 + jh:ih * 2 + jh + 1])
                # thres = 3 * mean = 3/1e6 * total
                dtot = smp.tile([P, 1], f32, tag="dtot")
                nc.vector.tensor_reduce(out=dtot[:], in_=dsum[:],
                                        axis=mybir.AxisListType.X, op=OP.add)
                tps = ptp.tile([1, 1], f32, tag="sps")
                nc.tensor.matmul(tps[:], lhsT=dtot[:], rhs=ones_col[:],
                                 start=True, stop=True)
                thres_sb = smp.tile([1, 1], f32, tag="th")
                nc.vector.tensor_scalar(thres_sb[:], tps[:], 3.0e-6, None,
                                        OP.mult)
                # broadcast thres to a [P,1] column via K=1 matmul
                thps = ptp.tile([P, 1], f32, tag="thps")
                nc.tensor.matmul(thps[:],
                                 lhsT=thres_sb[0:1, 0:1].to_broadcast([1, P]),
                                 rhs=ones11[:], start=True, stop=True)
                thres_col = smp.tile([P, 1], f32, tag="thc")
                nc.vector.tensor_copy(out=thres_col[:], in_=thps[:])

                # ------------- phase E: post-AllReduce epilogue -------------
                sums = bigp.tile([P, NH * D], f32, tag="sums")  # [c_lo, h*512+d]
                for h in range(NH):
                    nc.sync.dma_start(out=sums[:, h * D:(h + 1) * D],
                                      in_=shr_tab[h * P:(h + 1) * P, :])
                cntg = smp.tile([P, NH], f32, tag="cntg")
                nc.sync.dma_start(
                    out=cntg[:],
                    in_=shr_tab[CP:CP + 2, :].flatten().rearrange("(p h) -> p h", p=P))

                # recip of clamped counts, 1 Newton step
                clamp = smp.tile([P, NH], f32, tag="clamp")
                nc.vector.tensor_scalar(clamp[:], cntg[:], 1.0, None, OP.max)
                recip = smp.tile([P, NH], f32, tag="recip")
                nc.vector.reciprocal(recip[:], clamp[:])
                err1 = smp.tile([P, NH], f32, tag="err1")
                nc.vector.tensor_tensor(out=err1[:], in0=clamp[:], in1=recip[:],
                                        op=OP.mult)
                nc.vector.tensor_scalar(err1[:], err1[:], -1.0, 2.0, OP.mult,
                                        op1=OP.add)   # 2 - c*r
                nc.vector.tensor_tensor(out=recip[:], in0=recip[:], in1=err1[:],
                                        op=OP.mult)

                # new_center = (center + sums) * recip   [c_lo, h*512+d]
                ncsb = bigp.tile([P, NH * D], f32, tag="ncsb")
                nc.vector.tensor_tensor(out=ncsb[:], in0=cen_sb[:], in1=sums[:],
                                        op=OP.add)
                for h in range(NH):
                    nc.vector.tensor_scalar(
                        ncsb[:, h * D:(h + 1) * D], ncsb[:, h * D:(h + 1) * D],
                        recip[:, h:h + 1], None, OP.mult)

                # transpose to [d_lo, dh*1024 + c] (f32r) for the G matmuls
                ncT_r = bigp.tile([P, ND * CP], f32r, tag="ncT_r")
                for h in range(NH):
                    for dh in range(ND):
                        ps = ptp.tile([P, P], f32, tag="tps", bufs=3)
                        nc.tensor.transpose(
                            ps[:], ncsb[:, h * D + dh * P:h * D + dh * P + P],
                            ident[:])
                        nc.vector.tensor_copy(
                            out=ncT_r[:, dh * CP + h * P:dh * CP + h * P + P],
                            in_=ps[:])
                for h in range(NH):
                    rows = min(C - h * P, P)
                    if rows <= 0:
                        break
                    nc.sync.dma_start(out=outc_d[h * P:h * P + rows, :],
                                      in_=ncsb[:rows, h * D:h * D + D])

                # n2 for new centers
                n2n = smp.tile([P, NH], f32, tag="n2n")
                for h in range(NH):
                    nc.vector.tensor_tensor(out=sqt[:], in0=ncsb[:, h * D:h * D + D],
                                            in1=ncsb[:, h * D:h * D + D], op=OP.mult)
                    nc.vector.tensor_reduce(out=n2n[:, h:h + 1], in_=sqt[:],
                                            axis=mybir.AxisListType.X, op=OP.add)
                n2nb = bigp.tile([P, CP], f32, tag="n2nb")
                for h in range(NH):
                    ps = ptp.tile([P, P], f32, tag="tps", bufs=3)
                    nc.tensor.transpose(ps[:], n2n[:, h:h + 1].to_broadcast([P, P]),
                                        ident[:])
                    nc.vector.tensor_copy(out=n2nb[:, h * P:h * P + P], in_=ps[:])

                # hinge pass
                hsum = smp.tile([P, 16], f32, tag="hsum")
                nc.vector.memset(hsum[:], 0.0)
                for ih in range(NH):
                    m = ISIZES[ih]
                    for jh, (j0, jn) in enumerate(JSPANS):
                        g = ptp.tile([P, 512], f32, tag="gps", bufs=3)
                        for dh in range(ND):
                            nc.tensor.matmul(
                                g[:m, :jn],
                                lhsT=ncT_r[:, dh * CP + ih * P:dh * CP + ih * P + m],
                                rhs=ncT_r[:, dh * CP + j0:dh * CP + j0 + jn],
                                start=(dh == 0), stop=(dh == ND - 1))
                        nc.vector.tensor_scalar(
                            dtile[:m, :jn], g[:m, :jn], -2.0, n2n[:m, ih:ih + 1],
                            OP.mult, op1=OP.add)
                        nc.vector.tensor_tensor(out=dtile[:m, :jn],
                                                in0=dtile[:m, :jn],
                                                in1=n2nb[:m, j0:j0 + jn], op=OP.add)
                        nc.vector.tensor_scalar(dtile[:m, :jn], dtile[:m, :jn],
                                                0.0, None, OP.max)
                        nc.scalar.activation(dtile[:m, :jn], dtile[:m, :jn],
                                             ACT.Sqrt)
                        # hinge = relu(thres - dist), accumulate row sums
                        nc.scalar.activation(dtile[:m, :jn], dtile[:m, :jn],
                                             ACT.Relu, bias=thres_col[:m, 0:1],
                                             scale=-1.0,
                                             accum_out=hsum[:m, ih * 2 + jh:ih * 2 + jh + 1])
                htot = smp.tile([P, 1], f32, tag="htot")
                nc.vector.tensor_reduce(out=htot[:], in_=hsum[:],
                                        axis=mybir.AxisListType.X, op=OP.add)
                hps = ptp.tile([1, 1], f32, tag="sps")
                nc.tensor.matmul(hps[:], lhsT=htot[:], rhs=ones_col[:],
                                 start=True, stop=True)
                loss_sb = smp.tile([1, 1], f32, tag="loss")
                nc.vector.tensor_scalar(loss_sb[:], hps[:], 1.0e-6, None,
                                        OP.mult)
                nc.sync.dma_start(out=outp_d[:, :], in_=loss_sb[:])

    nc.compile()
    return nc


_NC = None


def kernel(features, center, labels):
    global _NC
    features = np.ascontiguousarray(features, dtype=np.float32)
    center = np.ascontiguousarray(center, dtype=np.float32)
    labels = np.ascontiguousarray(labels).astype(np.int32)

    if _NC is None:
        _NC = _build()

    in_maps = []
    for k in range(NCORES):
        in_maps.append({
            "feat": features[k * BL:(k + 1) * BL],
            "labels": labels[k * BL:(k + 1) * BL],
            "center": center,
        })
    res = run_bass_kernel_spmd(_NC, in_maps, core_ids=list(range(NCORES)))
    out = res.results[0]
    loss = np.float32(out["out_partial"][0, 0])
    new_center = out["out_center"].astype(np.float32)
    return np.array(loss, dtype=np.float32), new_center
